# revision 8
# baseline (speedup 1.0000x reference)
"""Multi-head cosine self-attention on 8 Trainium2 NeuronCores (Bass/Tile).

Problem: y = MHA(x) with L2-normalized q/k (cosine attention) and per-head
scaling sim / n**sigmoid(m);  x: [4, 2048, 1024], 16 heads of dim 64.

KEY REWRITE: there is no softmax, so attention is LINEAR:
    out_h = (q_hat @ k_hat^T / s_h) @ v  =  (q_hat / s_h) @ (k_hat^T @ v)
Associativity replaces the two O(n^2 d) stages ([2048x2048] sim per head)
with two tiny [64x64]-per-head GEMMs (KV = k_hat^T v, U = q_hat KV), a ~32x
FLOP reduction on the attention part.  The kernel becomes 4 dense
2048x1024x512 GEMM stages + cheap normalization.

Sharding: core c handles batch c//2 and head-group c%2 (8 heads = 512 of the
1024 q/k/v features).  Each core computes its partial output
(U_part @ Wo[rows]) in bf16; the host sums the two partials per batch in
f32 and adds bo.  No collectives.

Per-core pipeline (matmul operands bf16, fp32 PSUM accumulation).  The PE
executes in program order, so every matmul is emitted at a point where its
ACT/DVE/DMA producers have had time to finish:
  A. k, v natural [token-part, feat-free] per token tile; k row-norms per
     head (Pool square -> DVE tensor_reduce -> sqrt -> reciprocal ->
     stride-0-broadcast multiply); KV pair-block matmuls lag 2 tiles.
     x arrives as per-token-tile slabs so tile 0 is ready ~1.5us in.
  B. qT = (x Wq + bq)^T [feat-part, token-free], 2 heads per 128-tile;
     row norms via ones-block matmul -> [2, n]; per-chunk sqrt+reciprocal
     (the [2,n] reciprocal is partition-starved, so it is chunked and lags
     one pair); KV group closes + evicts during q_proj(0).
  C. UT = KV_h^T @ q_hatT_h, 2 heads packed via PE row+col tiling; pair 3
     is emitted after the last norm-apply so it never stalls.
  D. y = U @ Wo rows -> [2048, 1024] bf16 partial, DMA out per half-tile,
     interleaved with C chunk by chunk.
"""

import os
import sys

for _p in ("/opt/trn_rl_repo",):
    if os.path.isdir(_p) and _p not in sys.path:
        sys.path.insert(0, _p)

from contextlib import ExitStack

import ml_dtypes
import numpy as np

import concourse.bacc as bacc
import concourse.mybir as mybir
import concourse.tile as tile
from concourse import bass_utils
from concourse.bass import broadcast_tensor_aps

P = 128
F = 1024  # model dim
H = 16  # total heads
HD = 64  # head dim
G = 2  # head groups (tensor-parallel factor)
FG = F // G  # 512 features per core
PAIRS = FG // P  # 4 head-pairs per core
KT = F // P  # 8 contraction tiles for the q/k/v projections
NCORES = 8
F32 = mybir.dt.float32
BF = mybir.dt.bfloat16
AF = mybir.ActivationFunctionType
ALU = mybir.AluOpType


def build_core_program(nc, n=2048, has_bias=False):
    NT = n // P  # token tiles (16)
    NC = n // 512  # token chunks (4)
    KV_LAG = 2  # tiles of slack for the k-norm chain
    WARM_MMS = 3  # PE warm-up matmuls during the input-DMA window

    xt = nc.dram_tensor("xt", [P, NT, KT, P], BF, kind="ExternalInput").ap()
    wq = nc.dram_tensor("wq", [P, PAIRS, KT, P], BF, kind="ExternalInput").ap()
    wk = nc.dram_tensor("wk", [P, KT, FG], BF, kind="ExternalInput").ap()
    wv = nc.dram_tensor("wv", [P, KT, FG], BF, kind="ExternalInput").ap()
    wo = nc.dram_tensor("wo", [P, PAIRS, F], BF, kind="ExternalInput").ap()
    bqd = nc.dram_tensor("bq", [P, PAIRS], F32, kind="ExternalInput").ap()
    bkd = nc.dram_tensor("bk", [1, FG], BF, kind="ExternalInput").ap()
    bvd = nc.dram_tensor("bv", [1, FG], BF, kind="ExternalInput").ap()
    # cmsq[a, p] = (n ** sigmoid(m))**2 for local head 2p+a
    cmsq = nc.dram_tensor("cmsq", [2, PAIRS], F32, kind="ExternalInput").ap()
    cind = nc.dram_tensor("cind", [2, P], BF, kind="ExternalInput").ap()
    cblk = nc.dram_tensor("cblk", [P, 2], BF, kind="ExternalInput").ap()
    cone = nc.dram_tensor("cone", [1, P], BF, kind="ExternalInput").ap()
    cmsk = nc.dram_tensor("cmsk", [P, PAIRS * P], BF, kind="ExternalInput").ap()
    out = nc.dram_tensor("out", [n, F], BF, kind="ExternalOutput").ap()

    with tile.TileContext(nc) as tc, ExitStack() as ctx:
        const = ctx.enter_context(tc.tile_pool(name="const", bufs=1))
        persist = ctx.enter_context(tc.tile_pool(name="persist", bufs=1))
        work = ctx.enter_context(tc.tile_pool(name="work", bufs=1))
        ps = ctx.enter_context(tc.tile_pool(name="ps", bufs=1, space="PSUM"))

        # --- persistent activations -------------------------------------
        xts = persist.tile([P, NT, KT, P], BF)
        wqs = persist.tile([P, PAIRS, KT, P], BF)
        wks = persist.tile([P, KT, FG], BF)
        wvs = persist.tile([P, KT, FG], BF)
        wos = persist.tile([P, PAIRS, F], BF)
        qT = persist.tile([P, PAIRS, n], BF)  # q_hat^T, 2 heads per tile
        kh = persist.tile([P, NT, KT, HD], BF)  # k natural -> k_hat in place
        vn = persist.tile([P, NT, FG], BF)  # v natural
        ms = persist.tile([P, PAIRS, F], BF)  # M = KV_h @ Wo_h, d on part
        kvs = persist.tile([P, PAIRS * P], BF)  # KV^T pair blocks
        rec = persist.tile([P, NT, KT], F32)  # 1/||k|| per (token, head)

        # input DMAs.  Each dma_start costs ~0.6us of serialized HWDGE
        # dispatch, so inputs ship as FEW, LARGE transfers spread over three
        # queues (SP / DVE / ACT issue): phase-A tensors lead on each queue.
        # micro-slabs first so the very first matmul's operands land early
        nc.sync.dma_start(xts[:, 0, 0:1], xt[:, 0, 0:1])
        nc.sync.dma_start(wks[:, 0:1], wk[:, 0:1])
        nc.sync.dma_start(wvs[:, 0:1], wv[:, 0:1])
        nc.sync.dma_start(xts[:, 0, 1:KT], xt[:, 0, 1:KT])
        nc.sync.dma_start(wks[:, 1:4], wk[:, 1:4])
        nc.sync.dma_start(wvs[:, 1:4], wv[:, 1:4])
        nc.sync.dma_start(xts[:, 1:2], xt[:, 1:2])
        nc.sync.dma_start(xts[:, 2:3], xt[:, 2:3])
        nc.sync.dma_start(wks[:, 4:8], wk[:, 4:8])
        nc.sync.dma_start(wvs[:, 4:8], wv[:, 4:8])
        for ta, tb in ((3, 4), (4, 8), (8, 12), (12, 16)):
            nc.sync.dma_start(xts[:, ta:tb], xt[:, ta:tb])
        nc.sync.dma_start(wqs[:], wq)
        nc.sync.dma_start(wos[:], wo)

        # --- constants (consumed only by post-matmul ops) ----------------
        # on the gpsimd SWDGE queue so they never delay the HWDGE dispatch
        # of the critical weight/x transfers above
        ones_blk = const.tile([P, 2], BF)
        nc.gpsimd.dma_start(ones_blk[:], cblk)
        ind = const.tile([2, P], BF)
        nc.gpsimd.dma_start(ind[:], cind)
        zcol = const.tile([P, 1], F32)
        nc.any.memset(zcol[:], 0.0)
        bq_sb = const.tile([P, PAIRS], F32)
        nc.gpsimd.dma_start(bq_sb[:], bqd)
        cm_sb = const.tile([2, PAIRS], F32)
        nc.gpsimd.dma_start(cm_sb[:], cmsq)
        msk = const.tile([P, PAIRS * P], BF)
        nc.gpsimd.dma_start(msk[:], cmsk)
        if has_bias:
            ones_row = const.tile([1, P], BF)
            nc.gpsimd.dma_start(ones_row[:], cone)
            bk_sb = const.tile([1, FG], BF)
            nc.gpsimd.dma_start(bk_sb[:], bkd)
            bv_sb = const.tile([1, FG], BF)
            nc.gpsimd.dma_start(bv_sb[:], bvd)

        # --- PE warm-up: dummy matmuls on a zeroed tile bridge the input-
        # DMA window so the HAM clock-gate is at full rate (and the cost
        # model's pstate ramp is warm) when the first real matmul issues
        if WARM_MMS:
            wrm = const.tile([P, 512], BF)
            nc.vector.memset(wrm[:], 0.0)
            wscr = const.tile([P, 512], BF)
            wdr = nc.dram_tensor("wdr", [P, 512], BF, kind="Internal").ap()
            wp = ps.tile([P, 512], F32, tag="mm", bufs=7, name="wp")
            for i in range(WARM_MMS):
                nc.tensor.matmul(wp, wrm[:, 0:P], wrm[:], start=(i == 0),
                                 stop=(i == WARM_MMS - 1))
            # observable sink so the warm-up survives dead-code elimination
            nc.scalar.copy(wscr[:], wp)
            nc.gpsimd.dma_start(wdr, wscr[:])

        # ===== phase A: k/v natural projections + k norms + KV ===========
        # all 4 pair-accumulators live in ONE bank as a single accumulation
        # group: only the very first matmul passes start=True (it clears
        # has_written for the whole bank); per-element has_written then makes
        # each pair's first matmul overwrite and later ones accumulate.
        kvp = ps.tile([P, PAIRS * P], F32, tag="kv", bufs=1, name="kvp")

        def kv_mms(t):
            for p in range(PAIRS):
                first = t == 0 and p == 0
                last = t == NT - 1 and p == PAIRS - 1
                nc.tensor.matmul(kvp[:, p * P:(p + 1) * P],
                                 vn[:, t, p * P:(p + 1) * P],
                                 kh[:, t, 2 * p:2 * p + 2, :],
                                 start=first, stop=last,
                                 skip_group_check=not (first or last))

        def kv_tile(t):
            kp = ps.tile([P, FG], F32, tag="mm", bufs=7, name="kp")
            vp = ps.tile([P, FG], F32, tag="mm", bufs=7, name="vp")
            for kt in range(KT):
                nc.tensor.matmul(kp, xts[:, t, kt, :], wks[:, kt, :],
                                 start=(kt == 0),
                                 stop=(not has_bias and kt == KT - 1))
                nc.tensor.matmul(vp, xts[:, t, kt, :], wvs[:, kt, :],
                                 start=(kt == 0),
                                 stop=(not has_bias and kt == KT - 1))
            if has_bias:
                nc.tensor.matmul(kp, ones_row, bk_sb, start=False, stop=True)
                nc.tensor.matmul(vp, ones_row, bv_sb, start=False, stop=True)
            # KV matmuls lag KV_LAG tiles behind the projections so the PE
            # (in-order) never waits on the k-norm chain below
            if t >= KV_LAG:
                kv_mms(t - KV_LAG)
            khf = kh[:, t]  # [P, 8, 64] view
            nc.scalar.activation(khf, kp, AF.Identity, bias=zcol[:])
            nc.scalar.activation(vn[:, t, :], vp, AF.Identity, bias=zcol[:])
            # k-norm: sum of squares per head, rsqrt, apply in place
            sqk = work.tile([P, KT, HD], BF, tag="sqk", bufs=3, name="sqk")
            nc.gpsimd.tensor_tensor(sqk[:], khf, khf, ALU.mult)
            nc.vector.tensor_reduce(rec[:, t], sqk[:], mybir.AxisListType.X,
                                    ALU.add)
            nc.scalar.activation(rec[:, t], rec[:, t], AF.Sqrt, bias=zcol[:])
            nc.vector.reciprocal(rec[:, t], rec[:, t])
            a, b = broadcast_tensor_aps(khf, rec[:, t, :, None])
            nc.vector.tensor_tensor(khf, a, b, ALU.mult)

        for t in range(NT):
            kv_tile(t)

        # ============ phase B: qT projection + q norms ===================
        # chunk-granular software pipeline: proj(ft, ch) runs with
        # reduce(ft-1, ch) and apply(ft-2, ch) interleaved, so the PE
        # (in-order) never waits on the Pool/ACT/DVE norm chain.
        sqt = [None] * PAIRS
        rowt = [None] * PAIRS
        rowrt = [None] * PAIRS

        def q_proj_chunk(ft, ch):
            csl = slice(ch * 512, (ch + 1) * 512)
            pt = ps.tile([P, 512], F32, tag="mm", bufs=7, name="pt")
            for kt in range(KT):
                nc.tensor.matmul(pt, wqs[:, ft, kt, :],
                                 xts[:, 4 * ch:4 * ch + 4, kt, :],
                                 start=(kt == 0), stop=(kt == KT - 1))
            nc.scalar.activation(qT[:, ft, csl], pt, AF.Identity,
                                 bias=bq_sb[:, ft:ft + 1])
            # square on Pool (GpSimd), which is otherwise idle here
            if sqt[ft] is None:
                sqt[ft] = work.tile([P, n], BF, tag="sq", bufs=2, name="sq")
            nc.gpsimd.tensor_tensor(sqt[ft][:, csl], qT[:, ft, csl],
                                    qT[:, ft, csl], ALU.mult)

        def q_reduce_chunk(ft, ch):
            csl = slice(ch * 512, (ch + 1) * 512)
            if rowt[ft] is None:
                rowt[ft] = work.tile([2, n], F32, tag="row", bufs=2,
                                     name="row")
                rowrt[ft] = work.tile([2, n], BF, tag="rowr", bufs=2,
                                      name="rowr")
            nps = ps.tile([P, 512], F32, tag="mm", bufs=7, name="nps")
            nc.tensor.matmul(nps[0:2, :], ones_blk, sqt[ft][:, csl],
                             start=True, stop=True)
            nc.scalar.activation(rowt[ft][:, csl], nps[0:2, :], AF.Sqrt,
                                 bias=zcol[:2], scale=cm_sb[:, ft:ft + 1])
            with nc.allow_low_precision(
                    reason="1/(||q|| n^sig) consumed as bf16 matmul rhs"):
                nc.vector.reciprocal(rowrt[ft][:, csl], rowt[ft][:, csl])

        def q_apply_chunk(ft, ch):
            csl = slice(ch * 512, (ch + 1) * 512)
            bps = ps.tile([P, 512], F32, tag="mm", bufs=7, name="bps")
            nc.tensor.matmul(bps, ind, rowrt[ft][:, csl],
                             start=True, stop=True)
            nc.vector.tensor_tensor(qT[:, ft, csl], qT[:, ft, csl], bps,
                                    ALU.mult)

        for ft in range(PAIRS):
            for ch in range(NC):
                q_proj_chunk(ft, ch)
                if ft >= 1:
                    q_reduce_chunk(ft - 1, ch)
                if ft >= 2:
                    q_apply_chunk(ft - 2, ch)
            if ft == 0:
                # close the KV group: tiles 14/15's norm chains finished
                # during q_proj(0); then free the kv bank
                for t in range(NT - KV_LAG, NT):
                    kv_mms(t)
                # single whole-bank eviction, multiplied by the
                # diag-block mask: the cross-head blocks of each pair's
                # [128,128] become zero, so UT below is ONE full-K matmul
                # per (pair, chunk) with no tile_position packing
                nc.vector.tensor_tensor(kvs[:], kvp[:], msk[:], ALU.mult)

        # ====== phase C: M = KV_h @ Wo_h (y = U Wo = q_hat (KV Wo)) ======
        # kvs holds masked KV^T pair blocks, so M_pair = kvs_pair.T @ Wo_pair
        # in ONE matmul per (pair, half); the whole UT stage and its PSUM
        # evictions disappear, and the y projection reads q_hatT directly.
        def m_pair(p):
            for fc in range(2):
                fsl = slice(fc * 512, (fc + 1) * 512)
                mp = ps.tile([P, 512], F32, tag="mm", bufs=7, name="mp")
                nc.tensor.matmul(mp, kvs[:, p * P:(p + 1) * P],
                                 wos[:, p, fsl], start=True, stop=True)
                if fc == 0:
                    nc.vector.tensor_copy(ms[:, p, fsl], mp)
                else:
                    nc.scalar.copy(ms[:, p, fsl], mp)

        def y_tile(t):
            tsl = slice(t * P, (t + 1) * P)
            ys = work.tile([P, F], BF, tag="ys", bufs=3, name="ys")
            yps = []
            for fc in range(2):
                yp = ps.tile([P, 512], F32, tag="mm", bufs=7, name="yp")
                for p in range(PAIRS):
                    nc.tensor.matmul(yp, qT[:, p, tsl],
                                     ms[:, p, fc * 512:(fc + 1) * 512],
                                     start=(p == 0), stop=(p == PAIRS - 1))
                yps.append(yp)
            # the two halves evict concurrently (different banks, DVE + ACT)
            nc.vector.tensor_copy(ys[:, 0:512], yps[0])
            nc.scalar.copy(ys[:, 512:F], yps[1])
            if t == NT - 1:  # stream out the final tile per half
                nc.sync.dma_start(out[tsl, 0:512], ys[:, 0:512])
                nc.scalar.dma_start(out[tsl, 512:F], ys[:, 512:F])
            else:
                (nc.sync if t % 2 == 0 else nc.scalar).dma_start(out[tsl, :],
                                                                 ys[:])

        # epilogue: the M matmuls interleave as PE cover for the remaining
        # norm chain (reduce(3)/apply(2)/apply(3)); each chunk's applies
        # complete just ahead of that chunk's y tiles
        for ch in range(NC):
            q_reduce_chunk(3, ch)
            m_pair(ch)
        for ch in range(NC + 1):
            if ch < NC:
                q_apply_chunk(2, ch)
                q_apply_chunk(3, ch)
            if ch >= 1:
                for t in range(4 * (ch - 1), 4 * (ch - 1) + 4):
                    y_tile(t)
    return nc


_CACHE = {}


def get_nc(n=2048, has_bias=False):
    key = (n, has_bias)
    if key not in _CACHE:
        nc = bacc.Bacc("TRN2", target_bir_lowering=False, debug=False,
                       num_devices=NCORES)
        build_core_program(nc, n, has_bias)
        nc.compile()
        _CACHE[key] = nc
    return _CACHE[key]


BF_NP = ml_dtypes.bfloat16

_IND = np.zeros((2, P), np.float32)
_IND[0, :HD] = 1.0
_IND[1, HD:] = 1.0
_BLK = np.zeros((P, 2), np.float32)
_BLK[:HD, 0] = 1.0
_BLK[HD:, 1] = 1.0
_ONES = np.ones((1, P), np.float32)
# diag-block mask: zeroes the cross-head blocks of each pair's KV block
_MSK = np.zeros((P, PAIRS * P), np.float32)
for _p in range(PAIRS):
    _MSK[:HD, _p * P:_p * P + HD] = 1.0
    _MSK[HD:, _p * P + HD:(_p + 1) * P] = 1.0


def _warr(W, sl):
    return np.ascontiguousarray(
        np.asarray(W, np.float32)[:, sl].reshape(KT, P, FG)
        .transpose(1, 0, 2)).astype(BF_NP)


def _warr_ft(W, sl):
    return np.ascontiguousarray(
        np.asarray(W, np.float32)[:, sl].reshape(KT, P, PAIRS, P)
        .transpose(1, 2, 0, 3)).astype(BF_NP)


def make_in_maps(x, Wq, bq, Wk, bk, Wv, bv, Wo, bo, m):
    n = x.shape[1]
    NT = n // P
    sig = 1.0 / (1.0 + np.exp(-np.asarray(m, np.float64)))
    scale = np.float64(n) ** sig  # [16] per-head n^sigmoid(m)
    in_maps = []
    for c in range(NCORES):
        bi, g = divmod(c, 2)
        sl = slice(g * FG, (g + 1) * FG)
        hsc = scale[g * (H // G):(g + 1) * (H // G)]  # 8 local heads
        cm = (hsc ** 2).reshape(PAIRS, 2).T  # [2, PAIRS]
        xa = np.asarray(x[bi], np.float32)
        # xt[p, t, kt, j] = x[t*128 + j, kt*128 + p]
        xtile = np.ascontiguousarray(
            xa.reshape(NT, P, KT, P).transpose(3, 0, 2, 1)).astype(BF_NP)
        in_maps.append({
            "xt": xtile,
            "wq": _warr_ft(Wq, sl), "wk": _warr(Wk, sl), "wv": _warr(Wv, sl),
            "wo": np.ascontiguousarray(
                np.asarray(Wo, np.float32)[sl].reshape(PAIRS, P, F)
                .transpose(1, 0, 2)).astype(BF_NP),
            "bq": np.ascontiguousarray(
                np.asarray(bq, np.float32)[sl].reshape(PAIRS, P).T),
            "bk": np.asarray(bk, np.float32)[sl].reshape(1, FG).astype(BF_NP),
            "bv": np.asarray(bv, np.float32)[sl].reshape(1, FG).astype(BF_NP),
            "cmsq": np.ascontiguousarray(cm.astype(np.float32)),
            "cind": _IND.astype(BF_NP),
            "cblk": _BLK.astype(BF_NP),
            "cone": _ONES.astype(BF_NP),
            "cmsk": _MSK.astype(BF_NP),
        })
    return in_maps


def kernel(x, Wq, bq, Wk, bk, Wv, bv, Wo, bo, m, _trace=False):
    x = np.asarray(x, np.float32)
    b, n, f = x.shape
    has_bias = bool(np.any(np.asarray(bk)) or np.any(np.asarray(bv)))
    nc = get_nc(n, has_bias)
    in_maps = make_in_maps(x, Wq, bq, Wk, bk, Wv, bv, Wo, bo, m)
    res = bass_utils.run_bass_kernel_spmd(nc, in_maps,
                                          core_ids=list(range(NCORES)),
                                          trace=_trace)
    outs = [r["out"] for r in res.results]
    y = np.empty((b, n, f), np.float32)
    for bi in range(b):
        y[bi] = outs[2 * bi].astype(np.float32) + outs[2 * bi + 1]
    y += np.asarray(bo, np.float32).reshape(1, 1, f)
    kernel._last_results = res
    kernel._last_nc = nc
    return y


if __name__ == "__main__":
    nc = bacc.Bacc("TRN2", target_bir_lowering=False, debug=False,
                   num_devices=NCORES)
    build_core_program(nc, n=2048)
    print("build OK")


# revision 10
# speedup vs baseline: 1.0038x; 1.0038x over previous
"""Multi-head cosine self-attention on 8 Trainium2 NeuronCores (Bass/Tile).

Problem: y = MHA(x) with L2-normalized q/k (cosine attention) and per-head
scaling sim / n**sigmoid(m);  x: [4, 2048, 1024], 16 heads of dim 64.

KEY REWRITE: there is no softmax, so attention is LINEAR:
    out_h = (q_hat @ k_hat^T / s_h) @ v  =  (q_hat / s_h) @ (k_hat^T @ v)
Associativity replaces the two O(n^2 d) stages ([2048x2048] sim per head)
with two tiny [64x64]-per-head GEMMs (KV = k_hat^T v, U = q_hat KV), a ~32x
FLOP reduction on the attention part.  The kernel becomes 4 dense
2048x1024x512 GEMM stages + cheap normalization.

Sharding: core c handles batch c//2 and head-group c%2 (8 heads = 512 of the
1024 q/k/v features).  Each core computes its partial output
(U_part @ Wo[rows]) in bf16; the host sums the two partials per batch in
f32 and adds bo.  No collectives.

Per-core pipeline (matmul operands bf16, fp32 PSUM accumulation).  The PE
executes in program order, so every matmul is emitted at a point where its
ACT/DVE/DMA producers have had time to finish:
  A. k, v natural [token-part, feat-free] per token tile; k row-norms per
     head (Pool square -> DVE tensor_reduce -> sqrt -> reciprocal ->
     stride-0-broadcast multiply); KV pair-block matmuls lag 2 tiles.
     x arrives as per-token-tile slabs so tile 0 is ready ~1.5us in.
  B. qT = (x Wq + bq)^T [feat-part, token-free], 2 heads per 128-tile;
     row norms via ones-block matmul -> [2, n]; per-chunk sqrt+reciprocal
     (the [2,n] reciprocal is partition-starved, so it is chunked and lags
     one pair); KV group closes + evicts during q_proj(0).
  C. UT = KV_h^T @ q_hatT_h, 2 heads packed via PE row+col tiling; pair 3
     is emitted after the last norm-apply so it never stalls.
  D. y = U @ Wo rows -> [2048, 1024] bf16 partial, DMA out per half-tile,
     interleaved with C chunk by chunk.
"""

import os
import sys

for _p in ("/opt/trn_rl_repo",):
    if os.path.isdir(_p) and _p not in sys.path:
        sys.path.insert(0, _p)

from contextlib import ExitStack

import ml_dtypes
import numpy as np

import concourse.bacc as bacc
import concourse.mybir as mybir
import concourse.tile as tile
from concourse import bass_utils
from concourse.bass import broadcast_tensor_aps

P = 128
F = 1024  # model dim
H = 16  # total heads
HD = 64  # head dim
G = 2  # head groups (tensor-parallel factor)
FG = F // G  # 512 features per core
PAIRS = FG // P  # 4 head-pairs per core
KT = F // P  # 8 contraction tiles for the q/k/v projections
NCORES = 8
F32 = mybir.dt.float32
BF = mybir.dt.bfloat16
AF = mybir.ActivationFunctionType
ALU = mybir.AluOpType


def build_core_program(nc, n=2048, has_bias=False):
    NT = n // P  # token tiles (16)
    NC = n // 512  # token chunks (4)
    KV_LAG = 2  # tiles of slack for the k-norm chain
    WARM_MMS = 3  # PE warm-up matmuls during the input-DMA window

    xt = nc.dram_tensor("xt", [P, NT, KT, P], BF, kind="ExternalInput").ap()
    wq = nc.dram_tensor("wq", [P, PAIRS, KT, P], BF, kind="ExternalInput").ap()
    wk = nc.dram_tensor("wk", [P, KT, FG], BF, kind="ExternalInput").ap()
    wv = nc.dram_tensor("wv", [P, KT, FG], BF, kind="ExternalInput").ap()
    wo = nc.dram_tensor("wo", [P, PAIRS, F], BF, kind="ExternalInput").ap()
    bqd = nc.dram_tensor("bq", [P, PAIRS], F32, kind="ExternalInput").ap()
    bkd = nc.dram_tensor("bk", [1, FG], BF, kind="ExternalInput").ap()
    bvd = nc.dram_tensor("bv", [1, FG], BF, kind="ExternalInput").ap()
    # cmsq[a, p] = (n ** sigmoid(m))**2 for local head 2p+a
    cmsq = nc.dram_tensor("cmsq", [2, PAIRS], F32, kind="ExternalInput").ap()
    cind = nc.dram_tensor("cind", [2, P], BF, kind="ExternalInput").ap()
    cblk = nc.dram_tensor("cblk", [P, 2], BF, kind="ExternalInput").ap()
    cone = nc.dram_tensor("cone", [1, P], BF, kind="ExternalInput").ap()
    cmsk = nc.dram_tensor("cmsk", [P, PAIRS * P], BF, kind="ExternalInput").ap()
    out = nc.dram_tensor("out", [n, F], BF, kind="ExternalOutput").ap()

    with tile.TileContext(nc) as tc, ExitStack() as ctx:
        const = ctx.enter_context(tc.tile_pool(name="const", bufs=1))
        persist = ctx.enter_context(tc.tile_pool(name="persist", bufs=1))
        work = ctx.enter_context(tc.tile_pool(name="work", bufs=1))
        ps = ctx.enter_context(tc.tile_pool(name="ps", bufs=1, space="PSUM"))

        # --- persistent activations -------------------------------------
        xts = persist.tile([P, NT, KT, P], BF)
        wqs = persist.tile([P, PAIRS, KT, P], BF)
        wks = persist.tile([P, KT, FG], BF)
        wvs = persist.tile([P, KT, FG], BF)
        wos = persist.tile([P, PAIRS, F], BF)
        qT = persist.tile([P, PAIRS, n], BF)  # q_hat^T, 2 heads per tile
        kh = persist.tile([P, NT, KT, HD], BF)  # k natural -> k_hat in place
        vn = persist.tile([P, NT, FG], BF)  # v natural
        ms = persist.tile([P, PAIRS, F], BF)  # M = KV_h @ Wo_h, d on part
        kvs = persist.tile([P, PAIRS * P], BF)  # KV^T pair blocks
        rec = persist.tile([P, NT, KT], F32)  # 1/||k|| per (token, head)

        # input DMAs.  Each dma_start costs ~0.6us of serialized HWDGE
        # dispatch, so inputs ship as FEW, LARGE transfers spread over three
        # queues (SP / DVE / ACT issue): phase-A tensors lead on each queue.
        # micro-slabs first so the very first matmul's operands land early
        nc.sync.dma_start(xts[:, 0, 0:1], xt[:, 0, 0:1])
        nc.sync.dma_start(wks[:, 0:1], wk[:, 0:1])
        nc.sync.dma_start(wvs[:, 0:1], wv[:, 0:1])
        nc.sync.dma_start(xts[:, 0, 1:KT], xt[:, 0, 1:KT])
        nc.sync.dma_start(wks[:, 1:4], wk[:, 1:4])
        nc.sync.dma_start(wvs[:, 1:4], wv[:, 1:4])
        nc.sync.dma_start(xts[:, 1:2], xt[:, 1:2])
        nc.sync.dma_start(xts[:, 2:3], xt[:, 2:3])
        nc.sync.dma_start(wks[:, 4:8], wk[:, 4:8])
        nc.sync.dma_start(wvs[:, 4:8], wv[:, 4:8])
        for ta, tb in ((3, 4), (4, 8), (8, 12), (12, 16)):
            nc.sync.dma_start(xts[:, ta:tb], xt[:, ta:tb])
        nc.sync.dma_start(wqs[:], wq)
        nc.sync.dma_start(wos[:], wo)

        # --- constants (consumed only by post-matmul ops) ----------------
        # on the gpsimd SWDGE queue so they never delay the HWDGE dispatch
        # of the critical weight/x transfers above
        ones_blk = const.tile([P, 2], BF)
        nc.gpsimd.dma_start(ones_blk[:], cblk)
        ind = const.tile([2, P], BF)
        nc.gpsimd.dma_start(ind[:], cind)
        zcol = const.tile([P, 1], F32)
        nc.any.memset(zcol[:], 0.0)
        bq_sb = const.tile([P, PAIRS], F32)
        nc.gpsimd.dma_start(bq_sb[:], bqd)
        cm_sb = const.tile([2, PAIRS], F32)
        nc.gpsimd.dma_start(cm_sb[:], cmsq)
        msk = const.tile([P, PAIRS * P], BF)
        nc.gpsimd.dma_start(msk[:], cmsk)
        if has_bias:
            ones_row = const.tile([1, P], BF)
            nc.gpsimd.dma_start(ones_row[:], cone)
            bk_sb = const.tile([1, FG], BF)
            nc.gpsimd.dma_start(bk_sb[:], bkd)
            bv_sb = const.tile([1, FG], BF)
            nc.gpsimd.dma_start(bv_sb[:], bvd)

        # --- PE warm-up: dummy matmuls on a zeroed tile bridge the input-
        # DMA window so the HAM clock-gate is at full rate (and the cost
        # model's pstate ramp is warm) when the first real matmul issues
        if WARM_MMS:
            wrm = const.tile([P, 512], BF)
            nc.vector.memset(wrm[:], 0.0)
            wscr = const.tile([P, 512], BF)
            wdr = nc.dram_tensor("wdr", [P, 512], BF, kind="Internal").ap()
            wp = ps.tile([P, 512], F32, tag="mm", bufs=7, name="wp")
            for i in range(WARM_MMS):
                nc.tensor.matmul(wp, wrm[:, 0:P], wrm[:], start=(i == 0),
                                 stop=(i == WARM_MMS - 1))
            # observable sink so the warm-up survives dead-code elimination
            nc.scalar.copy(wscr[:], wp)
            nc.gpsimd.dma_start(wdr, wscr[:])

        # ===== phase A: k/v natural projections + k norms + KV ===========
        # all 4 pair-accumulators live in ONE bank as a single accumulation
        # group: only the very first matmul passes start=True (it clears
        # has_written for the whole bank); per-element has_written then makes
        # each pair's first matmul overwrite and later ones accumulate.
        kvp = ps.tile([P, PAIRS * P], F32, tag="kv", bufs=1, name="kvp")

        def kv_mms(t):
            for p in range(PAIRS):
                first = t == 0 and p == 0
                last = t == NT - 1 and p == PAIRS - 1
                nc.tensor.matmul(kvp[:, p * P:(p + 1) * P],
                                 vn[:, t, p * P:(p + 1) * P],
                                 kh[:, t, 2 * p:2 * p + 2, :],
                                 start=first, stop=last,
                                 skip_group_check=not (first or last))

        def kv_tile(t):
            kp = ps.tile([P, FG], F32, tag="mm", bufs=7, name="kp")
            vp = ps.tile([P, FG], F32, tag="mm", bufs=7, name="vp")
            # k matmuls lead within each weight half: wv arrives after wk,
            # and the in-order PE must not block ready k work behind v
            for half in range(2):
                kts = range(half * 4, half * 4 + 4)
                for kt in kts:
                    nc.tensor.matmul(kp, xts[:, t, kt, :], wks[:, kt, :],
                                     start=(kt == 0),
                                     stop=(not has_bias and kt == KT - 1))
                for kt in kts:
                    nc.tensor.matmul(vp, xts[:, t, kt, :], wvs[:, kt, :],
                                     start=(kt == 0),
                                     stop=(not has_bias and kt == KT - 1))
            if has_bias:
                nc.tensor.matmul(kp, ones_row, bk_sb, start=False, stop=True)
                nc.tensor.matmul(vp, ones_row, bv_sb, start=False, stop=True)
            # KV matmuls lag KV_LAG tiles behind the projections so the PE
            # (in-order) never waits on the k-norm chain below
            if t >= KV_LAG:
                kv_mms(t - KV_LAG)
            khf = kh[:, t]  # [P, 8, 64] view
            nc.scalar.activation(khf, kp, AF.Identity, bias=zcol[:])
            nc.scalar.activation(vn[:, t, :], vp, AF.Identity, bias=zcol[:])
            # k-norm: sum of squares per head, rsqrt, apply in place
            sqk = work.tile([P, KT, HD], BF, tag="sqk", bufs=3, name="sqk")
            nc.gpsimd.tensor_tensor(sqk[:], khf, khf, ALU.mult)
            nc.vector.tensor_reduce(rec[:, t], sqk[:], mybir.AxisListType.X,
                                    ALU.add)
            nc.scalar.activation(rec[:, t], rec[:, t], AF.Sqrt, bias=zcol[:])
            nc.vector.reciprocal(rec[:, t], rec[:, t])
            a, b = broadcast_tensor_aps(khf, rec[:, t, :, None])
            nc.vector.tensor_tensor(khf, a, b, ALU.mult)

        for t in range(NT):
            kv_tile(t)

        # ============ phase B: qT projection + q norms ===================
        # chunk-granular software pipeline: proj(ft, ch) runs with
        # reduce(ft-1, ch) and apply(ft-2, ch) interleaved, so the PE
        # (in-order) never waits on the Pool/ACT/DVE norm chain.
        sqt = [None] * PAIRS
        rowt = [None] * PAIRS
        rowrt = [None] * PAIRS

        def q_proj_chunk(ft, ch):
            csl = slice(ch * 512, (ch + 1) * 512)
            pt = ps.tile([P, 512], F32, tag="mm", bufs=7, name="pt")
            for kt in range(KT):
                nc.tensor.matmul(pt, wqs[:, ft, kt, :],
                                 xts[:, 4 * ch:4 * ch + 4, kt, :],
                                 start=(kt == 0), stop=(kt == KT - 1))
            nc.scalar.activation(qT[:, ft, csl], pt, AF.Identity,
                                 bias=bq_sb[:, ft:ft + 1])
            # square on Pool (GpSimd), which is otherwise idle here
            if sqt[ft] is None:
                sqt[ft] = work.tile([P, n], BF, tag="sq", bufs=2, name="sq")
            nc.gpsimd.tensor_tensor(sqt[ft][:, csl], qT[:, ft, csl],
                                    qT[:, ft, csl], ALU.mult)

        def q_reduce_chunk(ft, ch):
            csl = slice(ch * 512, (ch + 1) * 512)
            if rowt[ft] is None:
                rowt[ft] = work.tile([2, n], F32, tag="row", bufs=2,
                                     name="row")
                rowrt[ft] = work.tile([2, n], BF, tag="rowr", bufs=2,
                                      name="rowr")
            nps = ps.tile([P, 512], F32, tag="mm", bufs=7, name="nps")
            nc.tensor.matmul(nps[0:2, :], ones_blk, sqt[ft][:, csl],
                             start=True, stop=True)
            nc.scalar.activation(rowt[ft][:, csl], nps[0:2, :], AF.Sqrt,
                                 bias=zcol[:2], scale=cm_sb[:, ft:ft + 1])
            with nc.allow_low_precision(
                    reason="1/(||q|| n^sig) consumed as bf16 matmul rhs"):
                nc.vector.reciprocal(rowrt[ft][:, csl], rowt[ft][:, csl])

        def q_apply_chunk(ft, ch):
            csl = slice(ch * 512, (ch + 1) * 512)
            bps = ps.tile([P, 512], F32, tag="mm", bufs=7, name="bps")
            nc.tensor.matmul(bps, ind, rowrt[ft][:, csl],
                             start=True, stop=True)
            nc.vector.tensor_tensor(qT[:, ft, csl], qT[:, ft, csl], bps,
                                    ALU.mult)

        for ft in range(PAIRS):
            for ch in range(NC):
                q_proj_chunk(ft, ch)
                if ft >= 1:
                    q_reduce_chunk(ft - 1, ch)
                if ft >= 2:
                    q_apply_chunk(ft - 2, ch)
            if ft == 0:
                # close the KV group: tiles 14/15's norm chains finished
                # during q_proj(0); then free the kv bank
                for t in range(NT - KV_LAG, NT):
                    kv_mms(t)
                # single whole-bank eviction, multiplied by the
                # diag-block mask: the cross-head blocks of each pair's
                # [128,128] become zero, so UT below is ONE full-K matmul
                # per (pair, chunk) with no tile_position packing
                nc.vector.tensor_tensor(kvs[:], kvp[:], msk[:], ALU.mult)

        # ====== phase C: M = KV_h @ Wo_h (y = U Wo = q_hat (KV Wo)) ======
        # kvs holds masked KV^T pair blocks, so M_pair = kvs_pair.T @ Wo_pair
        # in ONE matmul per (pair, half); the whole UT stage and its PSUM
        # evictions disappear, and the y projection reads q_hatT directly.
        def m_pair(p):
            for fc in range(2):
                fsl = slice(fc * 512, (fc + 1) * 512)
                mp = ps.tile([P, 512], F32, tag="mm", bufs=7, name="mp")
                nc.tensor.matmul(mp, kvs[:, p * P:(p + 1) * P],
                                 wos[:, p, fsl], start=True, stop=True)
                if fc == 0:
                    nc.vector.tensor_copy(ms[:, p, fsl], mp)
                else:
                    nc.scalar.copy(ms[:, p, fsl], mp)

        def y_tile(t):
            tsl = slice(t * P, (t + 1) * P)
            ys = work.tile([P, F], BF, tag="ys", bufs=3, name="ys")
            yps = []
            for fc in range(2):
                yp = ps.tile([P, 512], F32, tag="mm", bufs=7, name="yp")
                for p in range(PAIRS):
                    nc.tensor.matmul(yp, qT[:, p, tsl],
                                     ms[:, p, fc * 512:(fc + 1) * 512],
                                     start=(p == 0), stop=(p == PAIRS - 1))
                yps.append(yp)
            # the two halves evict concurrently (different banks, DVE + ACT)
            nc.vector.tensor_copy(ys[:, 0:512], yps[0])
            nc.scalar.copy(ys[:, 512:F], yps[1])
            if t == NT - 1:  # final tile per half, same queue: transfers
                # chain back-to-back without the cross-queue bubble
                nc.sync.dma_start(out[tsl, 0:512], ys[:, 0:512])
                nc.sync.dma_start(out[tsl, 512:F], ys[:, 512:F])
            else:
                (nc.sync if t % 2 == 0 else nc.scalar).dma_start(out[tsl, :],
                                                                 ys[:])

        # epilogue: the M matmuls interleave as PE cover for the remaining
        # norm chain (reduce(3)/apply(2)/apply(3)); each chunk's applies
        # complete just ahead of that chunk's y tiles
        for ch in range(NC):
            q_reduce_chunk(3, ch)
            m_pair(ch)
        for ch in range(NC + 1):
            if ch < NC:
                q_apply_chunk(2, ch)
                q_apply_chunk(3, ch)
            if ch >= 1:
                for t in range(4 * (ch - 1), 4 * (ch - 1) + 4):
                    y_tile(t)
    return nc


_CACHE = {}


def get_nc(n=2048, has_bias=False):
    key = (n, has_bias)
    if key not in _CACHE:
        nc = bacc.Bacc("TRN2", target_bir_lowering=False, debug=False,
                       num_devices=NCORES)
        build_core_program(nc, n, has_bias)
        nc.compile()
        _CACHE[key] = nc
    return _CACHE[key]


BF_NP = ml_dtypes.bfloat16

_IND = np.zeros((2, P), np.float32)
_IND[0, :HD] = 1.0
_IND[1, HD:] = 1.0
_BLK = np.zeros((P, 2), np.float32)
_BLK[:HD, 0] = 1.0
_BLK[HD:, 1] = 1.0
_ONES = np.ones((1, P), np.float32)
# diag-block mask: zeroes the cross-head blocks of each pair's KV block
_MSK = np.zeros((P, PAIRS * P), np.float32)
for _p in range(PAIRS):
    _MSK[:HD, _p * P:_p * P + HD] = 1.0
    _MSK[HD:, _p * P + HD:(_p + 1) * P] = 1.0


def _warr(W, sl):
    return np.ascontiguousarray(
        np.asarray(W, np.float32)[:, sl].reshape(KT, P, FG)
        .transpose(1, 0, 2)).astype(BF_NP)


def _warr_ft(W, sl):
    return np.ascontiguousarray(
        np.asarray(W, np.float32)[:, sl].reshape(KT, P, PAIRS, P)
        .transpose(1, 2, 0, 3)).astype(BF_NP)


def make_in_maps(x, Wq, bq, Wk, bk, Wv, bv, Wo, bo, m):
    n = x.shape[1]
    NT = n // P
    sig = 1.0 / (1.0 + np.exp(-np.asarray(m, np.float64)))
    scale = np.float64(n) ** sig  # [16] per-head n^sigmoid(m)
    in_maps = []
    for c in range(NCORES):
        bi, g = divmod(c, 2)
        sl = slice(g * FG, (g + 1) * FG)
        hsc = scale[g * (H // G):(g + 1) * (H // G)]  # 8 local heads
        cm = (hsc ** 2).reshape(PAIRS, 2).T  # [2, PAIRS]
        xa = np.asarray(x[bi], np.float32)
        # xt[p, t, kt, j] = x[t*128 + j, kt*128 + p]
        xtile = np.ascontiguousarray(
            xa.reshape(NT, P, KT, P).transpose(3, 0, 2, 1)).astype(BF_NP)
        in_maps.append({
            "xt": xtile,
            "wq": _warr_ft(Wq, sl), "wk": _warr(Wk, sl), "wv": _warr(Wv, sl),
            "wo": np.ascontiguousarray(
                np.asarray(Wo, np.float32)[sl].reshape(PAIRS, P, F)
                .transpose(1, 0, 2)).astype(BF_NP),
            "bq": np.ascontiguousarray(
                np.asarray(bq, np.float32)[sl].reshape(PAIRS, P).T),
            "bk": np.asarray(bk, np.float32)[sl].reshape(1, FG).astype(BF_NP),
            "bv": np.asarray(bv, np.float32)[sl].reshape(1, FG).astype(BF_NP),
            "cmsq": np.ascontiguousarray(cm.astype(np.float32)),
            "cind": _IND.astype(BF_NP),
            "cblk": _BLK.astype(BF_NP),
            "cone": _ONES.astype(BF_NP),
            "cmsk": _MSK.astype(BF_NP),
        })
    return in_maps


def kernel(x, Wq, bq, Wk, bk, Wv, bv, Wo, bo, m, _trace=False):
    x = np.asarray(x, np.float32)
    b, n, f = x.shape
    has_bias = bool(np.any(np.asarray(bk)) or np.any(np.asarray(bv)))
    nc = get_nc(n, has_bias)
    in_maps = make_in_maps(x, Wq, bq, Wk, bk, Wv, bv, Wo, bo, m)
    res = bass_utils.run_bass_kernel_spmd(nc, in_maps,
                                          core_ids=list(range(NCORES)),
                                          trace=_trace)
    outs = [r["out"] for r in res.results]
    y = np.empty((b, n, f), np.float32)
    for bi in range(b):
        y[bi] = outs[2 * bi].astype(np.float32) + outs[2 * bi + 1]
    y += np.asarray(bo, np.float32).reshape(1, 1, f)
    kernel._last_results = res
    kernel._last_nc = nc
    return y


if __name__ == "__main__":
    nc = bacc.Bacc("TRN2", target_bir_lowering=False, debug=False,
                   num_devices=NCORES)
    build_core_program(nc, n=2048)
    print("build OK")


# revision 11
# speedup vs baseline: 1.0047x; 1.0008x over previous
"""Multi-head cosine self-attention on 8 Trainium2 NeuronCores (Bass/Tile).

Problem: y = MHA(x) with L2-normalized q/k (cosine attention) and per-head
scaling sim / n**sigmoid(m);  x: [4, 2048, 1024], 16 heads of dim 64.

KEY REWRITE: there is no softmax, so attention is LINEAR:
    out_h = (q_hat @ k_hat^T / s_h) @ v  =  (q_hat / s_h) @ (k_hat^T @ v)
Associativity replaces the two O(n^2 d) stages ([2048x2048] sim per head)
with two tiny [64x64]-per-head GEMMs (KV = k_hat^T v, U = q_hat KV), a ~32x
FLOP reduction on the attention part.  The kernel becomes 4 dense
2048x1024x512 GEMM stages + cheap normalization.

Sharding: core c handles batch c//2 and head-group c%2 (8 heads = 512 of the
1024 q/k/v features).  Each core computes its partial output
(U_part @ Wo[rows]) in bf16; the host sums the two partials per batch in
f32 and adds bo.  No collectives.

Per-core pipeline (matmul operands bf16, fp32 PSUM accumulation).  The PE
executes in program order, so every matmul is emitted at a point where its
ACT/DVE/DMA producers have had time to finish:
  A. k, v natural [token-part, feat-free] per token tile; k row-norms per
     head (Pool square -> DVE tensor_reduce -> sqrt -> reciprocal ->
     stride-0-broadcast multiply); KV pair-block matmuls lag 2 tiles.
     x arrives as per-token-tile slabs so tile 0 is ready ~1.5us in.
  B. qT = (x Wq + bq)^T [feat-part, token-free], 2 heads per 128-tile;
     row norms via ones-block matmul -> [2, n]; per-chunk sqrt+reciprocal
     (the [2,n] reciprocal is partition-starved, so it is chunked and lags
     one pair); KV group closes + evicts during q_proj(0).
  C. UT = KV_h^T @ q_hatT_h, 2 heads packed via PE row+col tiling; pair 3
     is emitted after the last norm-apply so it never stalls.
  D. y = U @ Wo rows -> [2048, 1024] bf16 partial, DMA out per half-tile,
     interleaved with C chunk by chunk.
"""

import os
import sys

for _p in ("/opt/trn_rl_repo",):
    if os.path.isdir(_p) and _p not in sys.path:
        sys.path.insert(0, _p)

from contextlib import ExitStack

import ml_dtypes
import numpy as np

import concourse.bacc as bacc
import concourse.mybir as mybir
import concourse.tile as tile
from concourse import bass_utils
from concourse.bass import broadcast_tensor_aps

P = 128
F = 1024  # model dim
H = 16  # total heads
HD = 64  # head dim
G = 2  # head groups (tensor-parallel factor)
FG = F // G  # 512 features per core
PAIRS = FG // P  # 4 head-pairs per core
KT = F // P  # 8 contraction tiles for the q/k/v projections
NCORES = 8
F32 = mybir.dt.float32
BF = mybir.dt.bfloat16
AF = mybir.ActivationFunctionType
ALU = mybir.AluOpType


def build_core_program(nc, n=2048, has_bias=False):
    NT = n // P  # token tiles (16)
    NC = n // 512  # token chunks (4)
    KV_LAG = 2  # tiles of slack for the k-norm chain
    WARM_MMS = 3  # PE warm-up matmuls during the input-DMA window

    xt = nc.dram_tensor("xt", [P, NT, KT, P], BF, kind="ExternalInput").ap()
    wq = nc.dram_tensor("wq", [P, PAIRS, KT, P], BF, kind="ExternalInput").ap()
    wk = nc.dram_tensor("wk", [P, KT, FG], BF, kind="ExternalInput").ap()
    wv = nc.dram_tensor("wv", [P, KT, FG], BF, kind="ExternalInput").ap()
    wo = nc.dram_tensor("wo", [P, PAIRS, F], BF, kind="ExternalInput").ap()
    bqd = nc.dram_tensor("bq", [P, PAIRS], F32, kind="ExternalInput").ap()
    bkd = nc.dram_tensor("bk", [1, FG], BF, kind="ExternalInput").ap()
    bvd = nc.dram_tensor("bv", [1, FG], BF, kind="ExternalInput").ap()
    # cmsq[a, p] = (n ** sigmoid(m))**2 for local head 2p+a
    cmsq = nc.dram_tensor("cmsq", [2, PAIRS], F32, kind="ExternalInput").ap()
    cind = nc.dram_tensor("cind", [2, P], BF, kind="ExternalInput").ap()
    cblk = nc.dram_tensor("cblk", [P, 2], BF, kind="ExternalInput").ap()
    cone = nc.dram_tensor("cone", [1, P], BF, kind="ExternalInput").ap()
    cmsk = nc.dram_tensor("cmsk", [P, PAIRS * P], BF, kind="ExternalInput").ap()
    out = nc.dram_tensor("out", [n, F], BF, kind="ExternalOutput").ap()

    with tile.TileContext(nc) as tc, ExitStack() as ctx:
        const = ctx.enter_context(tc.tile_pool(name="const", bufs=1))
        persist = ctx.enter_context(tc.tile_pool(name="persist", bufs=1))
        work = ctx.enter_context(tc.tile_pool(name="work", bufs=1))
        ps = ctx.enter_context(tc.tile_pool(name="ps", bufs=1, space="PSUM"))

        # --- persistent activations -------------------------------------
        xts = persist.tile([P, NT, KT, P], BF)
        wqs = persist.tile([P, PAIRS, KT, P], BF)
        wks = persist.tile([P, KT, FG], BF)
        wvs = persist.tile([P, KT, FG], BF)
        wos = persist.tile([P, PAIRS, F], BF)
        qT = persist.tile([P, PAIRS, n], BF)  # q_hat^T, 2 heads per tile
        kh = persist.tile([P, NT, KT, HD], BF)  # k natural -> k_hat in place
        vn = persist.tile([P, NT, FG], BF)  # v natural
        ms = persist.tile([P, PAIRS, F], BF)  # M = KV_h @ Wo_h, d on part
        kvs = persist.tile([P, PAIRS * P], BF)  # KV^T pair blocks
        rec = persist.tile([P, NT, KT], F32)  # 1/||k|| per (token, head)

        # input DMAs.  Each dma_start costs ~0.6us of serialized HWDGE
        # dispatch, so inputs ship as FEW, LARGE transfers spread over three
        # queues (SP / DVE / ACT issue): phase-A tensors lead on each queue.
        # micro-slabs first so the very first matmul's operands land early
        nc.sync.dma_start(xts[:, 0, 0:1], xt[:, 0, 0:1])
        nc.sync.dma_start(wks[:, 0:1], wk[:, 0:1])
        nc.sync.dma_start(wvs[:, 0:1], wv[:, 0:1])
        nc.sync.dma_start(xts[:, 0, 1:KT], xt[:, 0, 1:KT])
        nc.sync.dma_start(wks[:, 1:4], wk[:, 1:4])
        nc.sync.dma_start(wvs[:, 1:4], wv[:, 1:4])
        nc.sync.dma_start(xts[:, 1:2], xt[:, 1:2])
        nc.sync.dma_start(xts[:, 2:3], xt[:, 2:3])
        nc.sync.dma_start(wks[:, 4:8], wk[:, 4:8])
        nc.sync.dma_start(wvs[:, 4:8], wv[:, 4:8])
        for ta, tb in ((3, 4), (4, 8), (8, 12), (12, 16)):
            nc.sync.dma_start(xts[:, ta:tb], xt[:, ta:tb])
        nc.sync.dma_start(wqs[:], wq)
        nc.sync.dma_start(wos[:], wo)

        # --- constants (consumed only by post-matmul ops) ----------------
        # on the gpsimd SWDGE queue so they never delay the HWDGE dispatch
        # of the critical weight/x transfers above
        ones_blk = const.tile([P, 2], BF)
        nc.gpsimd.dma_start(ones_blk[:], cblk)
        ind = const.tile([2, P], BF)
        nc.gpsimd.dma_start(ind[:], cind)
        zcol = const.tile([P, 1], F32)
        nc.any.memset(zcol[:], 0.0)
        bq_sb = const.tile([P, PAIRS], F32)
        nc.gpsimd.dma_start(bq_sb[:], bqd)
        cm_sb = const.tile([2, PAIRS], F32)
        nc.gpsimd.dma_start(cm_sb[:], cmsq)
        msk = const.tile([P, PAIRS * P], BF)
        nc.gpsimd.dma_start(msk[:], cmsk)
        if has_bias:
            ones_row = const.tile([1, P], BF)
            nc.gpsimd.dma_start(ones_row[:], cone)
            bk_sb = const.tile([1, FG], BF)
            nc.gpsimd.dma_start(bk_sb[:], bkd)
            bv_sb = const.tile([1, FG], BF)
            nc.gpsimd.dma_start(bv_sb[:], bvd)

        # --- PE warm-up: dummy matmuls on a zeroed tile bridge the input-
        # DMA window so the HAM clock-gate is at full rate (and the cost
        # model's pstate ramp is warm) when the first real matmul issues
        if WARM_MMS:
            wrm = const.tile([P, 512], BF)
            nc.vector.memset(wrm[:], 0.0)
            wscr = const.tile([P, 512], BF)
            wdr = nc.dram_tensor("wdr", [P, 512], BF, kind="Internal").ap()
            wp = ps.tile([P, 512], F32, tag="mm", bufs=7, name="wp")
            for i in range(WARM_MMS):
                nc.tensor.matmul(wp, wrm[:, 0:P], wrm[:], start=(i == 0),
                                 stop=(i == WARM_MMS - 1))
            # observable sink so the warm-up survives dead-code elimination
            nc.scalar.copy(wscr[:], wp)
            nc.gpsimd.dma_start(wdr, wscr[:])

        # ===== phase A: k/v natural projections + k norms + KV ===========
        # all 4 pair-accumulators live in ONE bank as a single accumulation
        # group: only the very first matmul passes start=True (it clears
        # has_written for the whole bank); per-element has_written then makes
        # each pair's first matmul overwrite and later ones accumulate.
        kvp = ps.tile([P, PAIRS * P], F32, tag="kv", bufs=1, name="kvp")

        def kv_mms(t):
            for p in range(PAIRS):
                first = t == 0 and p == 0
                last = t == NT - 1 and p == PAIRS - 1
                nc.tensor.matmul(kvp[:, p * P:(p + 1) * P],
                                 vn[:, t, p * P:(p + 1) * P],
                                 kh[:, t, 2 * p:2 * p + 2, :],
                                 start=first, stop=last,
                                 skip_group_check=not (first or last))

        def kv_tile(t):
            kp = ps.tile([P, FG], F32, tag="mm", bufs=7, name="kp")
            vp = ps.tile([P, FG], F32, tag="mm", bufs=7, name="vp")
            # k matmuls lead within each weight half: wv arrives after wk,
            # and the in-order PE must not block ready k work behind v
            for half in range(2):
                kts = range(half * 4, half * 4 + 4)
                for kt in kts:
                    nc.tensor.matmul(kp, xts[:, t, kt, :], wks[:, kt, :],
                                     start=(kt == 0),
                                     stop=(not has_bias and kt == KT - 1))
                for kt in kts:
                    nc.tensor.matmul(vp, xts[:, t, kt, :], wvs[:, kt, :],
                                     start=(kt == 0),
                                     stop=(not has_bias and kt == KT - 1))
            if has_bias:
                nc.tensor.matmul(kp, ones_row, bk_sb, start=False, stop=True)
                nc.tensor.matmul(vp, ones_row, bv_sb, start=False, stop=True)
            # KV matmuls lag KV_LAG tiles behind the projections so the PE
            # (in-order) never waits on the k-norm chain below
            if t >= KV_LAG:
                kv_mms(t - KV_LAG)
            khf = kh[:, t]  # [P, 8, 64] view
            nc.scalar.activation(khf, kp, AF.Identity, bias=zcol[:])
            nc.scalar.activation(vn[:, t, :], vp, AF.Identity, bias=zcol[:])
            # k-norm: sum of squares per head, rsqrt, apply in place
            sqk = work.tile([P, KT, HD], BF, tag="sqk", bufs=3, name="sqk")
            nc.gpsimd.tensor_tensor(sqk[:], khf, khf, ALU.mult)
            nc.vector.tensor_reduce(rec[:, t], sqk[:], mybir.AxisListType.X,
                                    ALU.add)
            nc.scalar.activation(rec[:, t], rec[:, t], AF.Sqrt, bias=zcol[:])
            nc.vector.reciprocal(rec[:, t], rec[:, t])
            a, b = broadcast_tensor_aps(khf, rec[:, t, :, None])
            nc.vector.tensor_tensor(khf, a, b, ALU.mult)

        for t in range(NT):
            kv_tile(t)

        # ============ phase B: qT projection + q norms ===================
        # chunk-granular software pipeline: proj(ft, ch) runs with
        # reduce(ft-1, ch) and apply(ft-2, ch) interleaved, so the PE
        # (in-order) never waits on the Pool/ACT/DVE norm chain.
        sqt = [None] * PAIRS
        rowt = [None] * PAIRS
        rowrt = [None] * PAIRS

        def q_proj_chunk(ft, ch):
            csl = slice(ch * 512, (ch + 1) * 512)
            pt = ps.tile([P, 512], F32, tag="mm", bufs=7, name="pt")
            for kt in range(KT):
                nc.tensor.matmul(pt, wqs[:, ft, kt, :],
                                 xts[:, 4 * ch:4 * ch + 4, kt, :],
                                 start=(kt == 0), stop=(kt == KT - 1))
            nc.scalar.activation(qT[:, ft, csl], pt, AF.Identity,
                                 bias=bq_sb[:, ft:ft + 1])
            # square on Pool (GpSimd), which is otherwise idle here
            if sqt[ft] is None:
                sqt[ft] = work.tile([P, n], BF, tag="sq", bufs=2, name="sq")
            nc.gpsimd.tensor_tensor(sqt[ft][:, csl], qT[:, ft, csl],
                                    qT[:, ft, csl], ALU.mult)

        def q_reduce_chunk(ft, ch):
            csl = slice(ch * 512, (ch + 1) * 512)
            if rowt[ft] is None:
                rowt[ft] = work.tile([2, n], F32, tag="row", bufs=3,
                                     name="row")
                rowrt[ft] = work.tile([2, n], BF, tag="rowr", bufs=3,
                                      name="rowr")
            nps = ps.tile([P, 512], F32, tag="mm", bufs=7, name="nps")
            nc.tensor.matmul(nps[0:2, :], ones_blk, sqt[ft][:, csl],
                             start=True, stop=True)
            nc.scalar.activation(rowt[ft][:, csl], nps[0:2, :], AF.Sqrt,
                                 bias=zcol[:2], scale=cm_sb[:, ft:ft + 1])
            with nc.allow_low_precision(
                    reason="1/(||q|| n^sig) consumed as bf16 matmul rhs"):
                nc.vector.reciprocal(rowrt[ft][:, csl], rowt[ft][:, csl])

        def q_apply_chunk(ft, ch):
            csl = slice(ch * 512, (ch + 1) * 512)
            bps = ps.tile([P, 512], F32, tag="mm", bufs=7, name="bps")
            nc.tensor.matmul(bps, ind, rowrt[ft][:, csl],
                             start=True, stop=True)
            nc.vector.tensor_tensor(qT[:, ft, csl], qT[:, ft, csl], bps,
                                    ALU.mult)

        for ft in range(PAIRS):
            for ch in range(NC):
                q_proj_chunk(ft, ch)
                if ft >= 1:
                    q_reduce_chunk(ft - 1, ch)
                if ft == PAIRS - 1 and ch >= 1:
                    # the last pair's reduce pipelines into its own
                    # projection with a 1-chunk lag (no proj(4) to hide it)
                    q_reduce_chunk(PAIRS - 1, ch - 1)
                if ft >= 2:
                    q_apply_chunk(ft - 2, ch)
            if ft == 0:
                # close the KV group: tiles 14/15's norm chains finished
                # during q_proj(0); then free the kv bank
                for t in range(NT - KV_LAG, NT):
                    kv_mms(t)
                # single whole-bank eviction, multiplied by the
                # diag-block mask: the cross-head blocks of each pair's
                # [128,128] become zero, so UT below is ONE full-K matmul
                # per (pair, chunk) with no tile_position packing
                nc.vector.tensor_tensor(kvs[:], kvp[:], msk[:], ALU.mult)

        # ====== phase C: M = KV_h @ Wo_h (y = U Wo = q_hat (KV Wo)) ======
        # kvs holds masked KV^T pair blocks, so M_pair = kvs_pair.T @ Wo_pair
        # in ONE matmul per (pair, half); the whole UT stage and its PSUM
        # evictions disappear, and the y projection reads q_hatT directly.
        def m_pair(p):
            for fc in range(2):
                fsl = slice(fc * 512, (fc + 1) * 512)
                mp = ps.tile([P, 512], F32, tag="mm", bufs=7, name="mp")
                nc.tensor.matmul(mp, kvs[:, p * P:(p + 1) * P],
                                 wos[:, p, fsl], start=True, stop=True)
                if fc == 0:
                    nc.vector.tensor_copy(ms[:, p, fsl], mp)
                else:
                    nc.scalar.copy(ms[:, p, fsl], mp)

        def y_tile(t):
            tsl = slice(t * P, (t + 1) * P)
            ys = work.tile([P, F], BF, tag="ys", bufs=3, name="ys")
            yps = []
            for fc in range(2):
                yp = ps.tile([P, 512], F32, tag="mm", bufs=7, name="yp")
                for p in range(PAIRS):
                    nc.tensor.matmul(yp, qT[:, p, tsl],
                                     ms[:, p, fc * 512:(fc + 1) * 512],
                                     start=(p == 0), stop=(p == PAIRS - 1))
                yps.append(yp)
            # the two halves evict concurrently (different banks, DVE + ACT)
            nc.vector.tensor_copy(ys[:, 0:512], yps[0])
            nc.scalar.copy(ys[:, 512:F], yps[1])
            if t == NT - 1:  # final tile per half, same queue: transfers
                # chain back-to-back without the cross-queue bubble
                nc.sync.dma_start(out[tsl, 0:512], ys[:, 0:512])
                nc.sync.dma_start(out[tsl, 512:F], ys[:, 512:F])
            else:
                (nc.sync if t % 2 == 0 else nc.scalar).dma_start(out[tsl, :],
                                                                 ys[:])

        # epilogue: the M matmuls interleave as PE cover for the remaining
        # norm chain (reduce(3)/apply(2)/apply(3)); each chunk's applies
        # complete just ahead of that chunk's y tiles
        q_reduce_chunk(PAIRS - 1, NC - 1)
        for ch in range(NC):
            m_pair(ch)
        for ch in range(NC + 1):
            if ch < NC:
                q_apply_chunk(2, ch)
                q_apply_chunk(3, ch)
            if ch >= 1:
                for t in range(4 * (ch - 1), 4 * (ch - 1) + 4):
                    y_tile(t)
    return nc


_CACHE = {}


def get_nc(n=2048, has_bias=False):
    key = (n, has_bias)
    if key not in _CACHE:
        nc = bacc.Bacc("TRN2", target_bir_lowering=False, debug=False,
                       num_devices=NCORES)
        build_core_program(nc, n, has_bias)
        nc.compile()
        _CACHE[key] = nc
    return _CACHE[key]


BF_NP = ml_dtypes.bfloat16

_IND = np.zeros((2, P), np.float32)
_IND[0, :HD] = 1.0
_IND[1, HD:] = 1.0
_BLK = np.zeros((P, 2), np.float32)
_BLK[:HD, 0] = 1.0
_BLK[HD:, 1] = 1.0
_ONES = np.ones((1, P), np.float32)
# diag-block mask: zeroes the cross-head blocks of each pair's KV block
_MSK = np.zeros((P, PAIRS * P), np.float32)
for _p in range(PAIRS):
    _MSK[:HD, _p * P:_p * P + HD] = 1.0
    _MSK[HD:, _p * P + HD:(_p + 1) * P] = 1.0


def _warr(W, sl):
    return np.ascontiguousarray(
        np.asarray(W, np.float32)[:, sl].reshape(KT, P, FG)
        .transpose(1, 0, 2)).astype(BF_NP)


def _warr_ft(W, sl):
    return np.ascontiguousarray(
        np.asarray(W, np.float32)[:, sl].reshape(KT, P, PAIRS, P)
        .transpose(1, 2, 0, 3)).astype(BF_NP)


def make_in_maps(x, Wq, bq, Wk, bk, Wv, bv, Wo, bo, m):
    n = x.shape[1]
    NT = n // P
    sig = 1.0 / (1.0 + np.exp(-np.asarray(m, np.float64)))
    scale = np.float64(n) ** sig  # [16] per-head n^sigmoid(m)
    in_maps = []
    for c in range(NCORES):
        bi, g = divmod(c, 2)
        sl = slice(g * FG, (g + 1) * FG)
        hsc = scale[g * (H // G):(g + 1) * (H // G)]  # 8 local heads
        cm = (hsc ** 2).reshape(PAIRS, 2).T  # [2, PAIRS]
        xa = np.asarray(x[bi], np.float32)
        # xt[p, t, kt, j] = x[t*128 + j, kt*128 + p]
        xtile = np.ascontiguousarray(
            xa.reshape(NT, P, KT, P).transpose(3, 0, 2, 1)).astype(BF_NP)
        in_maps.append({
            "xt": xtile,
            "wq": _warr_ft(Wq, sl), "wk": _warr(Wk, sl), "wv": _warr(Wv, sl),
            "wo": np.ascontiguousarray(
                np.asarray(Wo, np.float32)[sl].reshape(PAIRS, P, F)
                .transpose(1, 0, 2)).astype(BF_NP),
            "bq": np.ascontiguousarray(
                np.asarray(bq, np.float32)[sl].reshape(PAIRS, P).T),
            "bk": np.asarray(bk, np.float32)[sl].reshape(1, FG).astype(BF_NP),
            "bv": np.asarray(bv, np.float32)[sl].reshape(1, FG).astype(BF_NP),
            "cmsq": np.ascontiguousarray(cm.astype(np.float32)),
            "cind": _IND.astype(BF_NP),
            "cblk": _BLK.astype(BF_NP),
            "cone": _ONES.astype(BF_NP),
            "cmsk": _MSK.astype(BF_NP),
        })
    return in_maps


def kernel(x, Wq, bq, Wk, bk, Wv, bv, Wo, bo, m, _trace=False):
    x = np.asarray(x, np.float32)
    b, n, f = x.shape
    has_bias = bool(np.any(np.asarray(bk)) or np.any(np.asarray(bv)))
    nc = get_nc(n, has_bias)
    in_maps = make_in_maps(x, Wq, bq, Wk, bk, Wv, bv, Wo, bo, m)
    res = bass_utils.run_bass_kernel_spmd(nc, in_maps,
                                          core_ids=list(range(NCORES)),
                                          trace=_trace)
    outs = [r["out"] for r in res.results]
    y = np.empty((b, n, f), np.float32)
    for bi in range(b):
        y[bi] = outs[2 * bi].astype(np.float32) + outs[2 * bi + 1]
    y += np.asarray(bo, np.float32).reshape(1, 1, f)
    kernel._last_results = res
    kernel._last_nc = nc
    return y


if __name__ == "__main__":
    nc = bacc.Bacc("TRN2", target_bir_lowering=False, debug=False,
                   num_devices=NCORES)
    build_core_program(nc, n=2048)
    print("build OK")
